# revision 6
# baseline (speedup 1.0000x reference)
"""Trainium2 Bass kernel v2 for nn_CausalAttention_50629074485540.

Causal MHA (B=2, T=2048, D=1024, H=16, hd=64) with ALiBi, tensor-parallel
over heads on 8 cores (2 heads/core), fp16 compute, host-side all-reduce.
Same math as the baseline (score matmul extended by 4 bias rows folding
ALiBi + a per-row softmax stabilizer; V carries a ones column so the PE
accumulates the softmax denominator for free).

v2 structural changes over the baseline (~215-226us -> ~190-199us on the
repetition-slope measurement):
  - causal narrowing: diagonal score tiles only compute columns i >= the
    j-tile start (saves ~10us PE + ~6us Act per core); the causal mask
    shrinks to one [128,128] inf-safe min against a precomputed clamp tile,
    kept on DVE (gpsimd per-op launch cost on HW is large; BIR also forbids
    gpsimd touching PSUM).
  - interleaved emission: the previous block's out-projection chunks and
    the next x-chunk's projection work (split into <=2-matmul closures) are
    woven between score tiles, so stalls of the s2 ring (2 PSUM bufs,
    ~1.2us round trip via exp on Act) are covered by independent PE work
    and vice versa.  Projections run up to 2 chunks ahead of need once the
    input stream has caught up.
  - the softmax normalize copies the attention accumulator PSUM->SBUF
    first (releases the PSUM bank ~2us earlier for the next block) and
    runs recip/broadcast/mul in f16 off-SBUF.
  - ext-row DMAs ride the Act/Pool DMA queues in parallel with the weight
    +x stream on SP (they were 4x3.16us serialized in front of everything
    and gated the first score matmul); the exp activation table is
    preloaded by a dummy activation at t=0; host pre-lays x/weights in
    partition-contiguous order so every DMA has dense descriptors; the
    tail block's out-projection borrows the idle s2 PSUM pool and Act for
    double-rate drain.
"""

import math
import sys
from contextlib import contextmanager

import numpy as np

for _p in ("/opt/trn_rl_repo", "/root/.axon_site/_ro/trn_rl_repo"):
    if _p not in sys.path:
        sys.path.append(_p)

import concourse.mybir as mybir
import concourse.tile as tile
from concourse import bacc, bass_utils
from concourse.bass import ts, ds
from concourse.masks import make_identity

F16 = mybir.dt.float16
F32 = mybir.dt.float32

B = 2
T = 2048
D = 1024
HD = 64
H = 16
N_CORES = 8
P = 128
KC = D // P          # 8 contraction chunks for projections
ECH = D // P         # 8 output-projection column chunks
CEXT = 68            # extended score contraction: 64 qk dims + 4 bias rows
# slot-0 window: j-tiles kept behind the diagonal block for the steep head.
# Steepest kept slope is 0.707^8 = 0.0625 (head 7): tiles beyond 3 j-tiles
# are >= 385 positions back => relative softmax weight < e^-20.
W0_TILES = 3


def core_heads(core):
    """Global head ids for (slot0, slot1): pair each steep-ALiBi head with a
    shallow one so slot 0 can run windowed attention on every core."""
    return core, 15 - core


def get_slopes(n):
    def pow2(n):
        start = 2 ** (-(2 ** (-(math.log2(n) - 3))))
        return [start * start**i for i in range(n)]
    if math.log2(n).is_integer():
        return pow2(n)
    c = 2 ** math.floor(math.log2(n))
    return pow2(c) + get_slopes(2 * c)[0::2][: n - c]


SEM_NS = 500


@contextmanager
def _hw_spec_patch():
    import concourse.hw_specs as hw

    saved = (hw.TRN2Spec.SEM_PROP_BASE_NS, hw.TRN2Spec.SEM_DELAY)
    hw.TRN2Spec.SEM_PROP_BASE_NS = SEM_NS
    hw.TRN2Spec.SEM_DELAY = SEM_NS
    try:
        yield
    finally:
        (hw.TRN2Spec.SEM_PROP_BASE_NS, hw.TRN2Spec.SEM_DELAY) = saved


def build_nc(reps=1, accum_out=False):
    with _hw_spec_patch():
        return _build_nc(reps, accum_out)


def _build_nc(reps=1, accum_out=False):
    BT = B * T
    TJ = T // P           # j-tiles per batch
    NCI = T // 512        # 512-wide i-chunks per batch
    TI = BT // 512        # 512-wide chunks over the full B*T axis

    nc = bacc.Bacc("TRN2", target_bir_lowering=False, debug=False,
                   enable_asserts=True, num_devices=N_CORES)

    # host pre-lays x as [TI, P, KC, 512] and weights as [P, KC, P] so every
    # DMA is contiguous per partition (dense descriptors)
    xT = nc.dram_tensor("xT", [TI, P, KC, 512], F16, kind="ExternalInput").ap()
    wq = nc.dram_tensor("wq", [P, KC, P], F16, kind="ExternalInput").ap()
    wk = nc.dram_tensor("wk", [P, KC, P], F16, kind="ExternalInput").ap()
    wv = nc.dram_tensor("wv", [P, KC, P], F16, kind="ExternalInput").ap()
    wo = nc.dram_tensor("wo", [P, D], F16, kind="ExternalInput").ap()
    qext = nc.dram_tensor("qext", [2, 4, BT], F16, kind="ExternalInput").ap()
    kext = nc.dram_tensor("kext", [2, 4, BT], F16, kind="ExternalInput").ap()
    yT = nc.dram_tensor("yT", [D, BT], F16, kind="ExternalOutput").ap()

    with tile.TileContext(nc) as tc:
        with tc.tile_pool(name="big", bufs=1) as big, \
             tc.tile_pool(name="ptiles", bufs=8) as ptiles, \
             tc.tile_pool(name="mtiles", bufs=4) as mtiles, \
             tc.tile_pool(name="ytiles", bufs=3) as ytiles, \
             tc.tile_pool(name="ntiles", bufs=6) as ntiles, \
             tc.tile_pool(name="vstage", bufs=2) as vstage_pool, \
             tc.tile_pool(name="pp", bufs=2, space="PSUM") as pp, \
             tc.tile_pool(name="ps_s", bufs=2, space="PSUM") as ps_s, \
             tc.tile_pool(name="ps_att", bufs=2, space="PSUM") as ps_att:

            # ---- persistent SBUF buffers ----
            xt_sb = big.tile([P, KC, BT], F16, tag="xt")
            wq_sb = big.tile([P, KC, P], F16, tag="wq")
            wk_sb = big.tile([P, KC, P], F16, tag="wk")
            wv_sb = big.tile([P, KC, P], F16, tag="wv")
            wo_sb = big.tile([P, D], F16, tag="wo")
            ident = big.tile([P, P], F16, tag="ident")
            # per-local-head Q~ / K~ [128, BT]: rows 0-63 head dims, 64-67 ext
            qt = [big.tile([P, BT], F16, tag=f"qt{h}", name=f"qt{h}")
                  for h in range(2)]
            kt = [big.tile([P, BT], F16, tag=f"kt{h}", name=f"kt{h}")
                  for h in range(2)]
            # V~ tiles: [j 128, b, tj, h, 65]; col 64 of each head = ones
            vt = big.tile([P, B, TJ, 2, HD + 1], F16, tag="vt")
            # tiny tile to preload the Exp activation table at t=0
            warm = big.tile([1, 8], F32, tag="warm")
            # [128,128] diagonal-alignment clamp mask: 60000 where i >= j
            # (valid), 0 where masked; applied with inf-safe min on DVE
            invm = big.tile([P, P], F16, tag="invm")

            make_identity(nc, ident[:])
            nc.gpsimd.memset(vt[:, :, :, :, HD], 1.0)
            nc.gpsimd.memset(warm[:], 0.0)
            nc.gpsimd.memset(invm[:], 60000.0)
            nc.gpsimd.affine_select(
                out=invm[:], in_=invm[:],
                compare_op=mybir.AluOpType.is_ge, fill=0.0,
                base=0, pattern=[[1, P]], channel_multiplier=-1)
            nc.scalar.activation(warm[:], warm[:],
                                 mybir.ActivationFunctionType.Exp,
                                 bias=0.0, scale=1.0)

            for _rep in range(reps):
                # ext rows on 4 separate queues so they land in parallel,
                # well before the first score matmul needs them
                nc.scalar.dma_start(qt[0][64:68, :], qext[0])
                nc.scalar.dma_start(qt[1][64:68, :], qext[1])
                nc.gpsimd.dma_start(kt[0][64:68, :], kext[0])
                nc.gpsimd.dma_start(kt[1][64:68, :], kext[1])
                nc.sync.dma_start(wq_sb[:], wq[:])
                nc.sync.dma_start(xt_sb[:, 0:2, ts(0, 512)], xT[0, :, 0:2])
                nc.sync.dma_start(xt_sb[:, 2:, ts(0, 512)], xT[0, :, 2:])
                nc.sync.dma_start(wk_sb[:], wk[:])
                nc.sync.dma_start(wv_sb[:], wv[:])
                nc.sync.dma_start(xt_sb[:, :, ts(1, 512)], xT[1])
                nc.sync.dma_start(wo_sb[:], wo[:])
                for ti in range(2, TI):
                    nc.sync.dma_start(xt_sb[:, :, ts(ti, 512)], xT[ti])

                # ---- task generators for interleaved emission ----
                def proj_tasks(ti):
                    """Yield fine-grained closures (<=2 matmuls each) for
                    projection chunk ti, so interleave injections between
                    score tiles never slip the s2 ring by more than ~400ns."""
                    state = {}

                    def mm_pair(w_sb, kc0):
                        if kc0 == 0:
                            state["ps"] = pp.tile([P, 512], F32, tag="proj",
                                                  name="ps")
                        ps = state["ps"]
                        for kc in (kc0, kc0 + 1):
                            nc.tensor.matmul(ps[:], w_sb[:, kc, :],
                                             xt_sb[:, kc, ts(ti, 512)],
                                             start=(kc == 0), stop=(kc == KC - 1))

                    def qk_fin(dst):
                        ps = state["ps"]
                        nc.vector.tensor_copy(dst[0][0:64, ts(ti, 512)], ps[0:64, :])
                        nc.vector.tensor_copy(dst[1][0:64, ts(ti, 512)], ps[64:128, :])

                    for w_sb, dst in ((wq_sb, qt), (wk_sb, kt)):
                        for kc0 in range(0, KC, 2):
                            yield lambda w_sb=w_sb, kc0=kc0: mm_pair(w_sb, kc0)
                        yield lambda dst=dst: qk_fin(dst)

                    for kc0 in range(0, KC, 2):
                        yield lambda kc0=kc0: mm_pair(wv_sb, kc0)

                    def v_fin():
                        vst = vstage_pool.tile([P, 512], F16, tag="vst", name="vst")
                        nc.vector.tensor_copy(vst[:], state["ps"][:])
                        state["vst"] = vst
                    yield v_fin

                    def tr_part(tt):
                        gt = ti * 4 + tt            # global 128-tile over B*T
                        b, tj = divmod(gt, TJ)
                        ps_tr = pp.tile([P, P], F16, tag="proj", name="ps_tr")
                        nc.tensor.transpose(ps_tr[:], state["vst"][:, ts(tt, P)],
                                            ident[:])
                        nc.vector.tensor_copy(
                            vt[:, b, tj, :, 0:HD],
                            ps_tr[:].rearrange("p (h c) -> p h c", h=2))
                    for tt in range(4):
                        yield lambda tt=tt: tr_part(tt)

                def out_tasks(merged, i0, tail):
                    """Yield closures for the out-projection of a finished
                    block: 8 matmul+copy chunks, then the output DMA.  The
                    tail block borrows the (idle by then) s2 pool for extra
                    PSUM turnover and rotates copies over 3 engines."""
                    ysb = ytiles.tile([P, ECH, 512], F16, tag="ysb", name="ysb")

                    def y_part(ec):
                        if tail and ec % 2 == 1:
                            y_ps = ps_s.tile([P, 2, 512], F32, tag="s",
                                             name="y_ps")[:, 0, :]
                        else:
                            y_ps = pp.tile([P, 512], F32, tag="proj",
                                           name="y_ps")[:]
                        nc.tensor.matmul(y_ps, wo_sb[:, ts(ec, P)], merged[:],
                                         start=True, stop=True)
                        # Pool cannot read PSUM, so these stay on DVE; the
                        # tail block (nothing left to overlap) borrows Act
                        if tail and ec % 2 == 1:
                            nc.scalar.copy(ysb[:, ec, :], y_ps)
                        else:
                            nc.vector.tensor_copy(ysb[:, ec, :], y_ps)
                    for ec in range(ECH):
                        yield lambda ec=ec: y_part(ec)

                    def dma_part():
                        yT_v = yT[:, ds(i0, 512)].rearrange("(e p) c -> p e c", p=P)
                        if accum_out:
                            nc.gpsimd.dma_start(yT_v, ysb[:],
                                                accum_op=mybir.AluOpType.add)
                        elif tail:
                            for eh in range(4):
                                nc.sync.dma_start(yT_v[:, ts(eh, 2), :],
                                                  ysb[:, ts(eh, 2), :])
                        else:
                            nc.sync.dma_start(yT_v, ysb[:])
                    yield dma_part

                def emit_attn(b, ci, extra):
                    """Emit one attention block, interleaving `extra` task
                    closures (previous block's out-proj + next proj chunk)
                    between score tiles so every engine ring's stall is
                    covered by independent work."""
                    i0 = b * T + ci * 512
                    ntj = 4 * ci + 4
                    merged = mtiles.tile([P, 512], F16, tag="merged", name="merged")
                    att = [ps_att.tile([HD + 1, 512], F32, tag="att", name="att")
                           for _ in range(2)]
                    ei = 0
                    # head slot 0 holds this core's steep-slope head (global
                    # head c, slope >= 0.0625): keys further than W0 j-tiles
                    # behind the diagonal carry relative weight < e^-20 and
                    # are skipped.  Slot 1 (head 15-c) runs the full range.
                    first0 = max(0, 4 * ci - W0_TILES)
                    for tj in range(ntj):
                        act0 = tj >= first0
                        heads = (0, 1) if act0 else (1,)
                        k = tj - 4 * ci
                        a = 128 * k if k > 0 else 0      # narrowed col offset
                        s2 = ps_s.tile([P, 2, 512], F32, tag="s", name="s2")
                        for h in heads:
                            nc.tensor.matmul(s2[:, h, a:512],
                                             kt[h][0:CEXT, ds(b * T + tj * P, P)],
                                             qt[h][0:CEXT, ds(i0 + a, 512 - a)],
                                             start=True, stop=True)
                        pt2 = ptiles.tile([P, 2, 512], F16, tag="pt", name="pt2")
                        if act0:
                            nc.scalar.activation(pt2[:, :, a:512], s2[:, :, a:512],
                                                 mybir.ActivationFunctionType.Exp,
                                                 bias=0.0, scale=0.125)
                        else:
                            nc.scalar.activation(pt2[:, 1, :], s2[:, 1, :],
                                                 mybir.ActivationFunctionType.Exp,
                                                 bias=0.0, scale=0.125)
                        if k >= 0:
                            # diagonal tile: zero j > i inside the leading
                            # [128,128] square (cols beyond are fully valid);
                            # min with the clamp mask is inf-safe and runs on
                            # DVE (gpsimd per-op launch cost is high on HW)
                            for h in range(2):
                                nc.vector.tensor_tensor(
                                    pt2[:, h, a:a + 128], pt2[:, h, a:a + 128],
                                    invm[:], mybir.AluOpType.min)
                        for h in heads:
                            nc.tensor.matmul(att[h][:, a:512], vt[:, b, tj, h, :],
                                             pt2[:, h, a:512],
                                             start=(tj == (first0 if h == 0 else 0)),
                                             stop=(tj == ntj - 1),
                                             skip_group_check=True)
                        if ei < len(extra):
                            extra[ei]()
                            ei += 1
                    while ei < len(extra):
                        extra[ei]()
                        ei += 1
                    for h in range(2):
                        # copy att to SBUF first: releases the PSUM
                        # accumulator ~2us earlier (the rest of the
                        # normalize chain runs off-SBUF in fast f16);
                        # h1's copy goes via Act so the two chains overlap
                        asb = ntiles.tile([HD + 1, 512], F16, tag="asb",
                                          name="asb")
                        with nc.allow_low_precision(reason="softmax in f16"):
                            nc.vector.tensor_copy(asb[:], att[h][:])
                            recip = ntiles.tile([1, 512], F16, tag="recip",
                                                name="recip")
                            nc.vector.reciprocal(recip[:], asb[HD:HD + 1, :])
                            rb = ntiles.tile([HD, 512], F16, tag="rb", name="rb")
                            nc.gpsimd.partition_broadcast(rb[:], recip[:])
                            nc.vector.tensor_mul(out=merged[ts(h, HD), :],
                                                 in0=asb[0:HD, :], in1=rb[:])
                    return merged, i0

                def zip_tasks(a, b):
                    out = []
                    for i in range(max(len(a), len(b))):
                        if i < len(a):
                            out.append(a[i])
                        if i < len(b):
                            out.append(b[i])
                    return out

                for t in proj_tasks(0):
                    t()
                pending = []          # out-proj tasks of the previous block
                next_proj = 1         # next projection chunk to emit
                blk = 0
                for b in range(B):
                    for ci in range(NCI):
                        # run projections up to 2 chunks ahead of need
                        # (just-in-time early on, where the x stream is
                        # still landing; 2-ahead once DMA has caught up)
                        ahead = blk + 1 if blk < 4 else min(blk + 2, TI - 1)
                        ptasks = []
                        while next_proj <= ahead:
                            ptasks.extend(proj_tasks(next_proj))
                            next_proj += 1
                        merged, i0 = emit_attn(b, ci, zip_tasks(pending, ptasks))
                        tail = (blk >= B * NCI - 2)
                        pending = list(out_tasks(merged, i0, tail))
                        blk += 1
                for t in pending:     # last block's out-projection
                    t()

    nc.compile()
    return nc


def make_core_inputs(x, Wq, Wk, Wv, Wo, core):
    """Build the fp16 input dict for one core. x: [B, T, D] fp32."""
    BT = B * T
    TI = BT // 512
    xT = x.reshape(BT, D).T.astype(np.float16)               # [D, BT]
    xT = np.ascontiguousarray(
        xT.reshape(KC, P, TI, 512).transpose(2, 1, 0, 3))    # [TI, 128, KC, 512]
    slopes = np.array(get_slopes(H), dtype=np.float64)
    g0, g1 = core_heads(core)
    cols = np.r_[HD * g0:HD * g0 + HD, HD * g1:HD * g1 + HD]

    def wlay(W):   # [D, 128-col head pair] -> [128, KC, 128]
        return np.ascontiguousarray(
            W[:, cols].astype(np.float16).reshape(KC, P, P).transpose(1, 0, 2))

    ins = {
        "xT": xT,
        "wq": wlay(Wq),
        "wk": wlay(Wk),
        "wv": wlay(Wv),
        "wo": np.ascontiguousarray(Wo[cols, :]).astype(np.float16),
    }
    pos = np.arange(T, dtype=np.float64)
    qe = np.zeros((2, 4, BT), np.float16)
    ke = np.zeros((2, 4, BT), np.float16)
    for h, g in enumerate(core_heads(core)):
        v = 8.0 * slopes[g] * (pos - 1024.0)       # j-side bias, fp16 2-split
        w = 8.0 * slopes[g] * (1024.0 - pos)       # i-side bias, fp16 2-split
        v1 = v.astype(np.float16)
        v2 = (v - v1.astype(np.float64)).astype(np.float16)
        w1 = w.astype(np.float16)
        w2 = (w - w1.astype(np.float64)).astype(np.float16)
        one = np.ones(T, np.float16)
        ke[h] = np.tile(np.stack([v1, v2, one, one]), (1, B))
        qe[h] = np.tile(np.stack([one, one, w1, w2]), (1, B))
    ins["qext"] = qe
    ins["kext"] = ke
    return ins


_NC_CACHE = {}


def _get_nc():
    if "nc" not in _NC_CACHE:
        _NC_CACHE["nc"] = build_nc()
    return _NC_CACHE["nc"]


def kernel(x, Wq, Wk, Wv, Wo):
    x = np.asarray(x, dtype=np.float32)
    Wq = np.asarray(Wq, dtype=np.float32)
    Wk = np.asarray(Wk, dtype=np.float32)
    Wv = np.asarray(Wv, dtype=np.float32)
    Wo = np.asarray(Wo, dtype=np.float32)
    assert x.shape == (B, T, D), x.shape

    nc = _get_nc()
    in_maps = [make_core_inputs(x, Wq, Wk, Wv, Wo, c) for c in range(N_CORES)]
    res = bass_utils.run_bass_kernel_spmd(nc, in_maps,
                                          core_ids=list(range(N_CORES)))
    acc = np.zeros((D, B * T), np.float32)
    for c in range(N_CORES):
        acc += res.results[c]["yT"].astype(np.float32)
    return np.ascontiguousarray(acc.T).reshape(B, T, D)
